# revision 53
# baseline (speedup 1.0000x reference)
"""Trainium2 Bass kernel for nn_G3DCrossAttention (B=2, C=512, L=2048, G=2048, H=8).

Algebraic structure (exact math): exp_p[g,b,:] = exp[b,g]*Wg[:,0]+bg is rank-1, so
k/v collapse to k = e*u_k + c_k, v = e*u_v + c_v.  The j-constant score shift
cancels in softmax, the attention output collapses per head to
    x_attn = w*u_v + c_v,   w_i = f_b(a_i),  a = x_seq @ M + a0,
with f_b(a) = d/da log Z_b(a),  Z_b(a) = sum_j exp(a*e_bj).  f_b is fitted on
HOST from the tiny `exp` input (logZ at 32 Chebyshev nodes -> series derivative
-> degree-9 monomial coefficients); the device evaluates f via Estrin's scheme
directly in the [8,T] head layout (t^2/t^4 + batch-1 linear terms on the scalar
engine, batch-0 + combines on vector), and applies it as one outer-product
matmul per 128-channel tile.

LN1 statistics are decomposed so the expensive part runs in the DMA window:
with y = y0 + uv.w (y0 = x_seq + c_v, rank-8 head structure), sum_c y and
sum_c y^2 reduce to xs-only matmuls (early) plus tiny wH-corrections:
  sum y   = sum y0 + sum_h uvs_h w_h
  sum y^2 = sum y0^2 + sum_h w_h (2 P_h + uvsq_h w_h),  P = uvT-blockdiag @ y0
All weight-only transforms (u_v/c_v, M, a0, b1'=b1+W1@be1, b2''=b2+be1, LN2
folded into Wo'=Wo*g2, bo'=bo+Wo@be2, s2=Wo'@1) are computed on HOST.
LN1 apply: x' = g1.(y-mu1).rstd1.  LN2 fold: po = Wo'@y2 + (-s2)(x)mu2, then
out = po .* bcast(rstd2) + bo'.

Sharding: data-parallel over L (LC=256 queries/core), full pipeline per core.
DMA: only sync+scalar have HW DGE queues; arbitration is per-PACKET, so
packet sizes are capped (max_dma_last_dim) to keep big weight loads from
starving the critical seq load.  Small constants ride ONE consolidated grid
(each dma_start costs ~2us of queue latency).  Queue plan: sync: xs chunks ->
W2 -> out; scalar: grid, uvh, W1, Wo, rows -> out; gpsimd idle.
"""

from contextlib import ExitStack

import numpy as np

import concourse.bass as bass
import concourse.tile as tile
from concourse import bacc, mybir
from concourse.bass_utils import run_bass_kernel_spmd

F32 = mybir.dt.float32
F32R = mybir.dt.float32r
FP16 = mybir.dt.float16
AF = mybir.ActivationFunctionType
OP = mybir.AluOpType

B, C, L, G, H = 2, 512, 2048, 2048, 8
D = C // H
NCORES = 8
LC = L // NCORES              # 256 queries per core
T = B * LC                    # 512 tokens per core, tau = b*LC + l
KC = C // 128                 # 4
KH = (4 * C) // 128           # 16
FP = 32                       # llo width of the packed a/w layout
SCALE = 1.0 / float(np.sqrt(D))
EPS = 1e-5
SCAL = 4.6                    # fit half-range in a units (|a|max ~ 4.43)
KD = 10                       # series length for f = (logZ)'
MN = 32                       # logZ sample nodes per batch (host)
NWARM = 3                     # PE warm-up matmuls while DMAs land

# ---- const grid column layout (f32 [128, GN]) -------------------------------
G_CBB = 0                     # [8, 2*KD]  monomial d coeffs per batch (rows 64-71)
G_CV = 2 * KD                 # [128, 4]   c_v per kt
G_B1 = 2 * KD + 4                 # [128, 16]  b1' per mt
G_B2 = 2 * KD + 20                # [128, 4]   b2'' per kt
G_G1 = 2 * KD + 24                # [128, 4]   g1 per kt
G_BO = 2 * KD + 28                # [128, 4]   bo' per mt
G_A0 = 2 * KD + 32                # [8, 1]     a0' col
G_CVS = 2 * KD + 33               # [1, 1]     sum(c_v)/C
G_PC = 2 * KD + 34                # [8, 1]     sum_{c in h} uv_c cv_c
G_UVQ2 = 2 * KD + 35              # [8, 1]     sum_{c in h} uv_c^2 / 2
G_F16 = 2 * KD + 36               # [128, 2]f32 = [.,4]fp16: uvs/C | 2/C
G_UVT = 2 * KD + 38               # [128,16]f32 = [.,32]fp16: uvT blockdiag cols
G_M16 = 2 * KD + 54               # [128,16]f32 = [.,32]fp16: M' cols
GN = 2 * KD + 70

RW_NS2 = 0                    # rows: -s2 [C] (tail only)
RW_BO = 512                   # rows: bo' [C]
RW_NCOL = 1024

TRACE = False
TRACE_KW = {}
LAST_RESULTS = None
_CACHE = None


def _host_consts():
    """Input-independent matrices for the host fit."""
    m = np.arange(MN)
    theta = np.pi * (2 * m + 1) / (2 * MN)
    xn = (SCAL * np.cos(theta)).astype(np.float64)          # nodes in a units
    F = np.zeros((KD, MN))
    for k in range(KD):
        F[k] = (2.0 / MN) * np.cos(k * theta)
    F[0] *= 0.5
    import numpy.polynomial.chebyshev as Ch
    DER = np.zeros((KD, KD))
    for k in range(KD):
        ck = np.zeros(KD)
        ck[k] = 1
        dd = Ch.chebder(ck)
        DER[:len(dd), k] = dd
    DM = (DER @ F) / SCAL                                   # [KD, MN]
    return xn, DM


_XN, _DM = _host_consts()


def _build():
    nc = bacc.Bacc(debug=False, num_devices=NCORES)

    # seq packed on host to [128, kt, b, l]: one DMA, 8KB lines
    seqp = nc.dram_tensor("seqp", [128, KC, B * LC], FP16, kind="ExternalInput")
    grid = nc.dram_tensor("grid", [128, GN], F32, kind="ExternalInput")
    rowsv = nc.dram_tensor("rowsv", [1, RW_NCOL], F32, kind="ExternalInput")
    uvha = nc.dram_tensor("uvha", [H, C], FP16, kind="ExternalInput")
    w1a = nc.dram_tensor("w1a", [128, KC, 4 * C], FP16, kind="ExternalInput")
    w2a = nc.dram_tensor("w2a", [128, KH, C], FP16, kind="ExternalInput")
    woa = nc.dram_tensor("woa", [128, KC, C], FP16, kind="ExternalInput")
    out_sl = nc.dram_tensor("out_sl", [128, KC, T], FP16, kind="ExternalOutput")

    with tile.TileContext(nc) as tc, ExitStack() as ctx:
        p_w = ctx.enter_context(tc.tile_pool(name="w", bufs=1))
        p_act = ctx.enter_context(tc.tile_pool(name="act", bufs=1))
        p_sm = ctx.enter_context(tc.tile_pool(name="sm", bufs=1))
        ps_mm = ctx.enter_context(tc.tile_pool(name="psmm", bufs=4, space="PSUM"))
        ps_xa = ctx.enter_context(tc.tile_pool(name="psxa", bufs=2, space="PSUM"))
        ps_st = ctx.enter_context(tc.tile_pool(name="psst", bufs=1, space="PSUM"))

        # ---- tiny on-chip constants (no DMA) -----------------------------
        wtile_f = p_sm.tile([128, T], F32, tag="warmf")
        nc.vector.memset(wtile_f[:], 0.0)
        wtile = p_sm.tile([128, T], F32R, tag="warm")
        nc.vector.tensor_copy(wtile[:], wtile_f[:])
        onesk = p_sm.tile([128, 1], FP16, tag="onesk")
        nc.vector.memset(onesk[:], 1.0 / C)
        onesf = p_sm.tile([1, 128], F32, tag="onesf")
        nc.vector.memset(onesf[:], 1.0)
        ones128 = p_sm.tile([1, 128], F32R, tag="ones128")
        nc.vector.tensor_copy(ones128[:], onesf[:])
        eps_col = p_sm.tile([1, 1], F32, tag="epsc")
        nc.vector.memset(eps_col[:], EPS)

        # ---- DMA loads ---------------------------------------------------
        # scalar HW queue: const grid, uvh, W1, rows
        gr = p_sm.tile([128, GN], F32, tag="gr")
        nc.scalar.dma_start(gr[:], grid[:])
        uvh = p_sm.tile([72, C], FP16, tag="uvh")
        nc.scalar.dma_start(uvh[64:72, :], uvha[:])
        w1s = p_w.tile([128, KC, 4 * C], FP16, tag="w1")
        nc.scalar.dma_start(w1s[:], w1a[:], max_dma_last_dim=2048)
        wos = p_w.tile([128, KC, C], FP16, tag="wo")
        nc.scalar.dma_start(wos[:], woa[:], max_dma_last_dim=2048)
        rowsr = p_sm.tile([1, RW_NCOL], F32R, tag="rowsr")
        nc.scalar.dma_start(rowsr[:], rowsv[:].bitcast(F32R))
        # sync HW queue: xs chunks first (4KB lines), then W2 behind them
        xs = p_w.tile([128, KC, B, LC], FP16, tag="xs")
        for ktp in range(2):
            nc.sync.dma_start(xs[:, 2 * ktp:2 * ktp + 2, :, :],
                              seqp[:, 2 * ktp:2 * ktp + 2, :]
                              .rearrange("p k (b l) -> p k b l", b=B))
        w2s = p_w.tile([128, KH, C], FP16, tag="w2")
        nc.sync.dma_start(w2s[:], w2a[:], max_dma_last_dim=2048)

        m16 = gr[:, G_M16:G_M16 + 16].bitcast(FP16)          # [128, 32]
        uvt16 = gr[:, G_UVT:G_UVT + 16].bitcast(FP16)        # [128, 32]
        f16v = gr[:, G_F16:G_F16 + 2].bitcast(FP16)          # [128, 4]
        uvs_col = f16v[64:72, 0:1]
        twoC_col = f16v[64:72, 1:2]

        # ---- PE warm-up while DMAs land ----------------------------------
        for i in range(NWARM):
            pw = ps_xa.tile([128, T], F32, tag="xa", name=f"warm{i}")
            nc.tensor.matmul(pw[0:8, :], wtile[:, 0:8], wtile[:], start=True, stop=True)

        # ---- a = x_seq @ M' (pre-scaled to t units); a0 added in the copy
        pa = ps_st.tile([72, T], F32, tag="st", name="pa")
        for kt in range(KC):
            nc.tensor.matmul(pa[64:72, :], m16[:, kt * 8:(kt + 1) * 8],
                             xs[:, kt, :, :],
                             start=(kt == 0), stop=(kt == KC - 1))
        tt_sb = p_sm.tile([72, T], F32, tag="tts")
        nc.scalar.activation(tt_sb[64:72, :], pa[64:72, :], AF.Identity,
                             bias=gr[64:72, G_A0:G_A0 + 1])

        t2f = p_sm.tile([72, T], F32, tag="t2f")
        nc.scalar.activation(t2f[64:72, :], tt_sb[64:72, :], AF.Square)
        t4f = p_sm.tile([72, T], F32, tag="t4f")
        nc.scalar.activation(t4f[64:72, :], t2f[64:72, :], AF.Square)
        # ---- LN1 stats, xs-only part (runs in the DMA window) ------------
        # st1: sum(y)/C at p0, sum(y^2)/C at p32, P = uvT@y0 at p64-71
        st1 = ps_st.tile([72, T], F32, tag="st", name="st1")
        for kt in range(KC):
            nc.tensor.matmul(st1[0:1, :], onesk[:], xs[:, kt, :, :],
                             start=(kt == 0), stop=False)
        for kt in range(KC):
            nc.tensor.matmul(st1[64:72, :], uvt16[:, kt * 8:(kt + 1) * 8],
                             xs[:, kt, :, :], start=(kt == 0), stop=(kt == KC - 1))
        def trickle(dep, nm):
            # keep-warm matmul dep-chained on an f-eval tile (f32 x f32)
            pw = ps_xa.tile([128, T], F32, tag="xa", name=f"trw{nm}")
            nc.tensor.matmul(pw[0:8, :], dep[64:72, 0:8], t2f[64:72, :],
                             start=True, stop=True)

        # ---- f via Estrin on per-batch monomial coeffs, [8, LC] slices ---
        # t2/t4 on the scalar engine, everything else on vector; no packing.
        wH = p_sm.tile([72, T], FP16, tag="wH")
        P_t = [p_sm.tile([72, T], F32, tag=f"P{j}", name=f"P{j}") for j in range(5)]
        cm_t = [p_sm.tile([72, T], F32, tag=f"cm{i}", name=f"cm{i}") for i in range(2)]
        Q_t = [p_sm.tile([72, T], F32, tag=f"Q{i}", name=f"Q{i}") for i in range(2)]
        u1 = p_sm.tile([72, T], F32, tag="u1")
        u2 = p_sm.tile([72, T], F32, tag="u2")
        v1 = p_sm.tile([72, T], F32, tag="v1")
        def dcol(b, k):
            return gr[64:72, G_CBB + b * KD + k:G_CBB + b * KD + k + 1]
        for j in (1, 0, 3, 2, 4):        # odd first: unblocks vector combines
            bs = slice(0, LC)            # batch 0 on vector
            nc.vector.tensor_scalar(P_t[j][64:72, bs], tt_sb[64:72, bs],
                                    dcol(0, 2 * j + 1), dcol(0, 2 * j),
                                    op0=OP.mult, op1=OP.add)
            bs = slice(LC, T)            # batch 1 on scalar (runs in parallel)
            nc.scalar.activation(P_t[j][64:72, bs], tt_sb[64:72, bs],
                                 AF.Identity, bias=dcol(1, 2 * j),
                                 scale=dcol(1, 2 * j + 1))
        xsq_t = []
        for kt in range(KC):
            xq = p_act.tile([128, T], FP16, tag="xq", bufs=4, name=f"xq{kt}")
            nc.scalar.activation(xq[:], xs[:, kt, :, :], AF.Square,
                                 bias=gr[:, G_CV + kt:G_CV + kt + 1])
            xsq_t.append(xq)
        for kt in range(KC):
            nc.tensor.matmul(st1[32:33, :], onesk[:], xsq_t[kt][:],
                             start=(kt == 0), stop=False)

        s8 = slice(64, 72)
        for i in range(2):
            nc.vector.tensor_mul(cm_t[i][s8, :], t2f[s8, :], P_t[2 * i + 1][s8, :])
            nc.vector.tensor_add(Q_t[i][s8, :], P_t[2 * i][s8, :], cm_t[i][s8, :])
            if i == 0:
                trickle(cm_t[0], "c0")
                trickle(Q_t[0], "q0")
        nc.vector.tensor_mul(u1[s8, :], t4f[s8, :], P_t[4][s8, :])
        nc.vector.tensor_add(u2[s8, :], Q_t[1][s8, :], u1[s8, :])
        nc.vector.tensor_mul(v1[s8, :], t4f[s8, :], u2[s8, :])
        nc.vector.tensor_add(wH[s8, :], Q_t[0][s8, :], v1[s8, :])

        # sum(y)/C += sum_h (uvs_h/C) w_h
        nc.tensor.matmul(st1[0:1, :], uvs_col, wH[64:72, :], start=False, stop=True)
        # sum(y^2)/C += (2/C) sum_h w_h [ (P_h + Pc_h) + (uvsq_h/2) w_h ]
        # (emitted before the y STTs so the LN1 rows chain overlaps them)
        u8 = p_sm.tile([72, T], F32, tag="u8")
        nc.vector.tensor_scalar(u8[64:72, :], st1[64:72, :],
                                gr[64:72, G_PC:G_PC + 1], None, op0=OP.add)
        v8 = p_sm.tile([72, T], F32, tag="v8")
        nc.vector.scalar_tensor_tensor(out=v8[64:72, :], in0=wH[64:72, :],
                                       scalar=gr[64:72, G_UVQ2:G_UVQ2 + 1],
                                       in1=u8[64:72, :], op0=OP.mult, op1=OP.add)
        z8 = p_sm.tile([72, T], FP16, tag="z8")
        nc.vector.tensor_mul(z8[64:72, :], v8[64:72, :], wH[64:72, :])
        mu1 = p_sm.tile([1, T], F32R, tag="mu1")
        nc.vector.tensor_scalar(mu1[:], st1[0:1, :], gr[0:1, G_CVS:G_CVS + 1],
                                None, op0=OP.add)
        musq1 = p_sm.tile([1, T], F32, tag="musq1")
        nc.vector.tensor_mul(musq1[:], mu1[:].bitcast(F32), mu1[:].bitcast(F32))
        y_t = []
        for kt in range(KC):
            xa = ps_xa.tile([128, T], F32, tag="xa", name=f"xa{kt}")
            nc.tensor.matmul(xa[:], uvh[64:72, kt * 128:(kt + 1) * 128],
                             wH[64:72, :], start=True, stop=True)
            yk = p_act.tile([128, T], FP16, tag="y", bufs=4, name=f"y{kt}")
            nc.vector.scalar_tensor_tensor(
                out=yk[:], in0=xa[:], scalar=gr[:, G_CV + kt:G_CV + kt + 1],
                in1=xs[:, kt, :, :], op0=OP.add, op1=OP.add)
            y_t.append(yk)
        nc.tensor.matmul(st1[32:33, :], twoC_col, z8[64:72, :], start=False, stop=True)

        def ln_rows(stA, stB, ph, cvs=None, want_mu=False):
            mu = None
            if want_mu:
                mu = p_sm.tile([1, T], F32R, tag="mu", bufs=2, name=f"mu{ph}")
                if cvs is not None:
                    nc.vector.tensor_scalar(mu[:], stA, cvs, None, op0=OP.add)
                else:
                    nc.vector.tensor_copy(mu[:], stA)
            musq = p_sm.tile([1, T], F32, tag="lnr", bufs=6, name=f"musq{ph}")
            if want_mu:
                nc.vector.tensor_mul(musq[:], mu[:].bitcast(F32),
                                     mu[:].bitcast(F32))
            else:
                nc.scalar.activation(musq[:], stA, AF.Square,
                                     bias=cvs if cvs is not None else 0.0)
            var = p_sm.tile([1, T], F32, tag="lnr", bufs=6, name=f"var{ph}")
            nc.vector.tensor_sub(var[:], stB, musq[:])
            std = p_sm.tile([1, T], F32R, tag="lnr", bufs=6, name=f"std{ph}")
            nc.scalar.activation(std[:], var[:], AF.Sqrt, bias=eps_col[:])
            rstd_f = p_sm.tile([1, T], F32, tag="rstdf", bufs=2, name=f"rstdf{ph}")
            nc.vector.reciprocal_approx_fast(rstd_f[:], std[:].bitcast(F32))
            rstd = p_sm.tile([1, T], F32R, tag="rstd", bufs=2, name=f"rstd{ph}")
            nc.vector.tensor_copy(rstd[:], rstd_f[:])
            return mu, rstd, std

        # ---- LN1 apply -> x' = g1.(y - mu1).rstd1 ------------------------
        var1 = p_sm.tile([1, T], F32, tag="var1")
        nc.vector.tensor_sub(var1[:], st1[32:33, :], musq1[:])
        std1 = p_sm.tile([1, T], F32R, tag="std1")
        nc.scalar.activation(std1[:], var1[:], AF.Sqrt, bias=eps_col[:])
        rstd1f = p_sm.tile([1, T], F32, tag="rstd1f")
        nc.vector.reciprocal_approx_fast(rstd1f[:], std1[:].bitcast(F32))
        rstd1 = p_sm.tile([1, T], F32R, tag="rstd1")
        nc.vector.tensor_copy(rstd1[:], rstd1f[:])
        mu1b = ps_xa.tile([128, T], F32, tag="xa", name="mu1b")
        nc.tensor.matmul(mu1b[:], ones128[:], mu1[:], start=True, stop=True)
        pwv = ps_mm.tile([128, T], F32, tag="mm", name="pwv")
        nc.tensor.matmul(pwv[0:8, :], var1[0:1, 0:8], wtile_f[0:1, :],
                         start=True, stop=True)
        r1b = ps_xa.tile([128, T], F32, tag="xa", name="r1b")
        nc.tensor.matmul(r1b[:], ones128[:], rstd1[:], start=True, stop=True)
        x_t = []
        for kt in range(KC):
            yc = p_act.tile([128, T], FP16, tag="tx", bufs=2, name=f"yc{kt}")
            nc.vector.tensor_sub(yc[:], y_t[kt][:], mu1b[:])
            xo = p_act.tile([128, T], FP16, tag="x", bufs=4, name=f"x{kt}")
            nc.vector.scalar_tensor_tensor(
                out=xo[:], in0=yc[:], scalar=gr[:, G_G1 + kt:G_G1 + kt + 1],
                in1=r1b[:], op0=OP.mult, op1=OP.mult)
            x_t.append(xo)

        # ---- FFN1: h = relu(W1 @ x' + b1') -------------------------------
        h_t = []
        for mt in range(KH):
            pf = ps_mm.tile([128, T], F32, tag="mm", name=f"pf1{mt}")
            for kt in range(KC):
                nc.tensor.matmul(pf[:], w1s[:, kt, mt * 128:(mt + 1) * 128],
                                 x_t[kt][:], start=(kt == 0), stop=(kt == KC - 1))
            hm = p_act.tile([128, T], FP16, tag="h", bufs=KH, name=f"h{mt}")
            nc.scalar.activation(hm[:], pf[:], AF.Relu,
                                 bias=gr[:, G_B1 + mt:G_B1 + mt + 1])
            h_t.append(hm)

        # ---- FFN2 + residual -> y2 = x' + W2@h + b2'' --------------------
        y2_t = []
        st2 = ps_st.tile([33, T], F32, tag="st", name="st2")
        sq_t = []
        for mt in range(KC):
            pf = ps_mm.tile([128, T], F32, tag="mm", name=f"pf2{mt}")
            for kh in range(KH):
                nc.tensor.matmul(pf[:], w2s[:, kh, mt * 128:(mt + 1) * 128],
                                 h_t[kh][:], start=(kh == 0), stop=(kh == KH - 1))
            y2 = p_act.tile([128, T], FP16, tag="y2", bufs=4, name=f"y2{mt}")
            nc.vector.scalar_tensor_tensor(
                out=y2[:], in0=x_t[mt][:], scalar=gr[:, G_B2 + mt:G_B2 + mt + 1],
                in1=pf[:], op0=OP.add, op1=OP.add)
            y2_t.append(y2)
            nc.tensor.matmul(st2[0:1, :], onesk[:], y2[:],
                             start=(mt == 0), stop=(mt == KC - 1))
            sq = p_act.tile([128, T], FP16, tag="sq", bufs=4, name=f"sqb{mt}")
            nc.vector.tensor_mul(sq[:], y2[:], y2[:])
            sq_t.append(sq)

        # ---- LN2 folded into output projection ---------------------------
        # out = (Wo'@y2 + (-s2)(x)mu2) .* bcast(rstd2) + bo'
        for mt in range(KC):
            nc.tensor.matmul(st2[32:33, :], onesk[:], sq_t[mt][:],
                             start=(mt == 0), stop=(mt == KC - 1))
        po_t = [ps_mm.tile([128, T], F32, tag="mm", name=f"po{mt}")
                for mt in range(KC)]
        for kt in range(KC - 1):
            for mt in range(KC):
                nc.tensor.matmul(po_t[mt][:],
                                 wos[:, kt, mt * 128:(mt + 1) * 128],
                                 y2_t[kt][:], start=(kt == 0), stop=False)
        mu2, rstd2, std2 = ln_rows(st2[0:1, :], st2[32:33, :], "b", want_mu=True)
        rb_ps = ps_xa.tile([128, T], F32, tag="xa", name="rb")
        nc.tensor.matmul(rb_ps[:], ones128[:], rstd2[:], start=True, stop=True)
        rb_sb = p_sm.tile([128, T], F32, tag="rbs")
        nc.vector.tensor_copy(rb_sb[:], rb_ps[:])
        for mt in range(KC):
            kt = KC - 1
            nc.tensor.matmul(po_t[mt][:],
                             wos[:, kt, mt * 128:(mt + 1) * 128],
                             y2_t[kt][:], start=False, stop=False)
            nc.tensor.matmul(po_t[mt][:],
                             rowsr[0:1, RW_BO + mt * 128:RW_BO + (mt + 1) * 128],
                             std2[:], start=False, stop=False)
            nc.tensor.matmul(po_t[mt][:],
                             rowsr[0:1, RW_NS2 + mt * 128:RW_NS2 + (mt + 1) * 128],
                             mu2[:], start=False, stop=True)
            om = p_act.tile([128, T], FP16, tag="om", bufs=4, name=f"om{mt}")
            nc.vector.tensor_mul(om[:], po_t[mt][:], rb_sb[:])
            seng = nc.sync if mt % 2 == 0 else nc.scalar
            seng.dma_start(out_sl[:, mt, :], om[:])

    nc.compile()
    return nc


def _host_pack(inputs):
    f32 = lambda x: np.asarray(x, dtype=np.float32)
    Wq, Wk, Wv, Wo = (f32(inputs[k]) for k in ("Wq", "Wk", "Wv", "Wo"))
    W1, W2 = f32(inputs["W1"]), f32(inputs["W2"])
    Wg = f32(inputs["Wg"])[:, 0]
    bg, bq, bv, b1, b2, bo = (f32(inputs[k]) for k in ("bg", "bq", "bv", "b1", "b2", "bo"))
    g1, be1, g2, be2 = (f32(inputs[k]) for k in ("g1", "beta1", "g2", "beta2"))
    expv = np.asarray(inputs["exp"], dtype=np.float64)

    u_k = Wk @ Wg
    u_v = Wv @ Wg
    c_v = Wv @ bg + bv
    M = np.zeros((C, H), np.float32)
    a0 = np.zeros(H, np.float32)
    for h in range(H):
        ukh = u_k[h * D:(h + 1) * D]
        M[:, h] = Wq[h * D:(h + 1) * D, :].T @ ukh
        a0[h] = bq[h * D:(h + 1) * D] @ ukh
    Mp = M * (SCALE / SCAL)
    a0p = a0 * (SCALE / SCAL)
    uvH = np.zeros((H, C), np.float32)
    for h in range(H):
        uvH[h, h * D:(h + 1) * D] = u_v[h * D:(h + 1) * D]
    Wop = Wo * g2[None, :]
    bop = bo + Wo @ be2
    s2 = Wop.sum(1)
    b1p = b1 + W1 @ be1
    b2p = b2 + be1

    # f_b fit from the tiny `exp` input -> monomial coeffs (Estrin on device)
    import numpy.polynomial.chebyshev as Ch
    lnz = np.zeros((MN, B))
    for b in range(B):
        lnz[:, b] = np.log(np.exp(_XN[:, None] * expv[b][None, :]).sum(1))
    ck = _DM @ lnz                                           # [KD, B]
    dmono = np.zeros((KD, B))
    for b in range(B):
        dd = Ch.cheb2poly(ck[:, b])
        dmono[:len(dd), b] = dd

    gr = np.zeros((128, GN), np.float32)
    for b in range(B):
        gr[64:72, G_CBB + b * KD:G_CBB + (b + 1) * KD] = dmono[:, b]
    for kt in range(KC):
        gr[:, G_CV + kt] = c_v[kt * 128:(kt + 1) * 128]
        gr[:, G_B2 + kt] = b2p[kt * 128:(kt + 1) * 128]
        gr[:, G_G1 + kt] = g1[kt * 128:(kt + 1) * 128]
        gr[:, G_BO + kt] = bop[kt * 128:(kt + 1) * 128]
    for mt in range(KH):
        gr[:, G_B1 + mt] = b1p[mt * 128:(mt + 1) * 128]
    gr[64:72, G_A0] = a0p
    gr[0, G_CVS] = c_v.sum() / C
    uv_cv = uvH * c_v[None, :]                               # [H, C]
    gr[64:72, G_PC] = uv_cv.sum(1)
    gr[64:72, G_UVQ2] = (uvH ** 2).sum(1) / 2.0
    tmp = np.zeros((128, 2), np.float32)
    tv = tmp.view(np.float16)
    tv[64:72, 0] = (uvH.sum(1) / C).astype(np.float16)
    tv[64:72, 1] = np.float16(2.0 / C)
    gr[:, G_F16:G_F16 + 2] = tmp
    tmp = np.zeros((128, 16), np.float32)
    tv = tmp.view(np.float16)                                # [128, 32]
    for kt in range(KC):
        tv[:, kt * 8:(kt + 1) * 8] = uvH[:, kt * 128:(kt + 1) * 128].T
    gr[:, G_UVT:G_UVT + 16] = tmp
    tmp = np.zeros((128, 16), np.float32)
    tv = tmp.view(np.float16)
    for kt in range(KC):
        tv[:, kt * 8:(kt + 1) * 8] = Mp[kt * 128:(kt + 1) * 128, :]
    gr[:, G_M16:G_M16 + 16] = tmp

    rowsv = np.zeros((1, RW_NCOL), np.float32)
    rowsv[0, RW_NS2:RW_NS2 + C] = -s2
    rowsv[0, RW_BO:RW_BO + C] = bop

    def packw(W):
        # W: [M, K] -> [128, K/128, M] with w[p, kt, m] = W[m, kt*128+p]
        M, K = W.shape
        return np.ascontiguousarray(
            W.astype(np.float16).T.reshape(K // 128, 128, M).transpose(1, 0, 2))
    return {
        "grid": gr,
        "rowsv": rowsv,
        "uvha": np.asarray(uvH, dtype=np.float16),
        "w1a": packw(W1),
        "w2a": packw(W2),
        "woa": packw(Wop),
    }


def kernel(**inputs):
    global _CACHE, LAST_RESULTS
    if _CACHE is None:
        _CACHE = _build()
    nc = _CACHE

    base = _host_pack(inputs)
    seq = np.asarray(inputs["seq"], dtype=np.float16)
    # pre-pack seq to device layout [128, kt, b, l]
    seq4 = seq.reshape(B, KC, 128, L).transpose(2, 1, 0, 3)      # [128, KC, B, L]
    in_maps = []
    for c in range(NCORES):
        m = dict(base)
        m["seqp"] = np.ascontiguousarray(
            seq4[:, :, :, c * LC:(c + 1) * LC]).reshape(128, KC, B * LC)
        in_maps.append(m)

    res = run_bass_kernel_spmd(nc, in_maps, list(range(NCORES)), trace=TRACE,
                               **TRACE_KW)
    LAST_RESULTS = res
    out = np.empty((B, C, L), np.float32)
    for c in range(NCORES):
        o = res.results[c]["out_sl"].astype(np.float32)      # [128, KC, B*LC]
        o = o.reshape(128, KC, B, LC).transpose(2, 1, 0, 3)  # [B, KC, 128, LC]
        out[:, :, c * LC:(c + 1) * LC] = o.reshape(B, C, LC)
    return out
